# revision 57
# baseline (speedup 1.0000x reference)
"""Trainium2 Bass kernel for nn_MultiHeadAttention (conv-projected MHA).

Reference (B=4, C=512, L=2048, H=8, D=64):
    qc = conv1d_same(q, wq)            # [B, C, L]
    qh = qc.reshape(B, -1, H, D).transpose(0,2,1,3)
    ... attn = softmax(qh @ kh / D); out = attn @ vh
    out -> [B, C, L] -> conv1d_same(out, fc)

KEY LAYOUT FACT: the row-major reshape of [C, L=2048] to [n=2048, H=8, D=64]
means attention-sequence index n = c*4 + l//512, head/feature = l%512 =
h*64 + d.  So heads slice along L, and the 2048 attention positions are
(channel c, quarter j=l//512) pairs.

Sharding: 8 cores = (batch, L-half).  A core owns output columns
l' in [half*1024, half*1024+1024), i.e. attention groups j in {2*half,
2*half+1} for all heads, plus the final conv's halo columns l'=qlo-1 /
qhi+1.  Each halo column is a single (h, d) feature of the 512 positions
of one j-group; both boundary columns (l'=1023: j=1,h=7,d=63 and
l'=1024: j=2,h=0,d=0) are computed redundantly on every core (uniform
SPMD program) and masked by host-provided 0/1 scalars.

On-core dataflow (per batch):
  - q conv, k conv in TRANSPOSED orientation ([l, c]; lhsT = input chunk,
    rhs = host-pretransposed W^T[cin, cout])
  - v conv in NORMAL orientation ([c, l]) evacuated into 65-wide per-
    (j', h) slots with a ones column (softmax denominator trick), bias
    included
  - per (h, jj): scores^T[n'(16 chunks of c'), c] = kT.T @ qT (K = D = 64),
    exp on ScalarE (PSUM->SBUF, 4 banks per instruction),
    O^T[65, c] accumulated over 16 n'-chunks (row 64 = denominators),
    PE-transpose 128-blocks -> [c, 65], per-partition reciprocal +
    tensor_scalar multiply -> attn_out[c, l'] (bf16)
  - fc conv in NORMAL orientation from attn_out [C, 1026] -> out [C, 1024]
"""

import os

import numpy as np
import ml_dtypes

B, C, L = 4, 512, 2048
H, D = 8, 64
NCORES = 8
HALF = L // 2
QW = HALF + 2            # attn_out buffer cols: halo + 1024 + halo
NJ = 4                   # j groups total
KC = 16                  # n' chunks (j' * 4 + c'chunk)
CIN_CH = 4
COUT_CH = 4
VSLOT = D + 1            # 65
QIN_W = 1026 + 130 + 130  # own window + two boundary segments

BF16 = ml_dtypes.bfloat16

_CACHE = {}
_LAST_IN_MAPS = None
_LAST_RESULTS = None

# Boundary columns, computed uniformly on every core:
#   l'=1023 = (j=1, h=7, d=63): qT slot 8 (l 896..1024), rows 64..128
#   l'=1024 = (j=2, h=0, d=0):  qT slot 9 (l 1024..1152), rows 0..64
VARIANTS = (
    {"h": 7, "d": 63, "qslot": 8, "qrow0": 64, "col": 0},
    {"h": 0, "d": 0, "qslot": 9, "qrow0": 0, "col": QW - 1},
)


def _build(flags):
    use_qb, use_kb, use_vb, use_fb = flags
    import concourse.bass as bass
    import concourse.bacc as bacc
    import concourse.tile as tile
    from concourse import mybir
    from concourse.masks import make_identity
    from contextlib import ExitStack

    f32 = mybir.dt.float32
    bf16 = mybir.dt.bfloat16

    def bcast_rows(ap, nrows):
        return bass.AP(tensor=ap.tensor, offset=ap.offset,
                       ap=[[0, nrows]] + [list(d) for d in ap.ap[1:]])

    nc = bacc.Bacc("TRN2", target_bir_lowering=False, debug=False,
                   num_devices=NCORES)

    q_in_d = nc.dram_tensor("q_in", [C, QIN_W], bf16, kind="ExternalInput").ap()
    # k/v conv inputs are the core's OWN l-half only (+1 halo col each side);
    # the two cores of a batch exchange conv results via pairwise AllGather.
    k_in_d = nc.dram_tensor("k_in", [C, 1026], bf16, kind="ExternalInput").ap()
    v_in_d = nc.dram_tensor("v_in", [C, 1026], bf16, kind="ExternalInput").ap()
    k_src = nc.dram_tensor("k_src", [128, 8, C], bf16).ap()
    k_gath = nc.dram_tensor("k_gath", [2, 128, 8, C], bf16).ap()
    v_src = nc.dram_tensor("v_src", [128, CIN_CH, 16 * VSLOT], bf16).ap()
    v_gath = nc.dram_tensor("v_gath", [2, 128, CIN_CH, 16 * VSLOT], bf16).ap()
    wq_d = nc.dram_tensor("wq", [3, C, C], bf16, kind="ExternalInput").ap()
    wk_d = nc.dram_tensor("wk", [3, C, C], bf16, kind="ExternalInput").ap()
    wv_d = nc.dram_tensor("wv", [3, C, C], bf16, kind="ExternalInput").ap()
    wfc_d = nc.dram_tensor("wfc", [3, C, C], bf16, kind="ExternalInput").ap()
    mab_d = nc.dram_tensor("mab", [1, 2], f32, kind="ExternalInput").ap()
    qb_d = kb_d = vb_d = fb_d = None
    if use_qb:
        qb_d = nc.dram_tensor("qb", [1, C], f32, kind="ExternalInput").ap()
    if use_kb:
        kb_d = nc.dram_tensor("kb", [1, C], f32, kind="ExternalInput").ap()
    if use_vb:
        vb_d = nc.dram_tensor("vb", [128, CIN_CH], f32, kind="ExternalInput").ap()
    if use_fb:
        fb_d = nc.dram_tensor("fb", [128, CIN_CH], f32, kind="ExternalInput").ap()
    out_d = nc.dram_tensor("out", [C, HALF], f32, kind="ExternalOutput").ap()

    dbg = bool(os.environ.get("BASS_DEBUG_DUMP"))
    if dbg:
        dbg_kt = nc.dram_tensor("dbg_kt", [128, KC, C], bf16,
                                kind="ExternalOutput").ap()
        dbg_qt = nc.dram_tensor("dbg_qt", [128, 10, C], bf16,
                                kind="ExternalOutput").ap()
        dbg_vs = nc.dram_tensor("dbg_vs", [128, CIN_CH, 32 * VSLOT], bf16,
                                kind="ExternalOutput").ap()
        dbg_exp = nc.dram_tensor("dbg_exp", [128, KC, 512], bf16,
                                 kind="ExternalOutput").ap()
        dbg_o = nc.dram_tensor("dbg_o", [VSLOT, 512], bf16,
                               kind="ExternalOutput").ap()
        dbg_ao = nc.dram_tensor("dbg_ao", [128, CIN_CH, QW], bf16,
                                kind="ExternalOutput").ap()

    with tile.TileContext(nc) as tc, ExitStack() as ctx:
        consts = ctx.enter_context(tc.tile_pool(name="consts", bufs=1))
        # PSUM budget (8 banks): shared (convs/fc/transposes) 2 +
        # scores 4x1 (bf16) + o 2 = 8
        shared_ps = ctx.enter_context(
            tc.tile_pool(name="shared_ps", bufs=2, space="PSUM"))
        scores_ps = ctx.enter_context(
            tc.tile_pool(name="scores_ps", bufs=2, space="PSUM"))
        o_ps = ctx.enter_context(tc.tile_pool(name="o_ps", bufs=2, space="PSUM"))
        conv_ps = shared_ps
        tp_ps = shared_ps
        exp_pool = ctx.enter_context(tc.tile_pool(name="exp_pool", bufs=3))
        o_sb_pool = ctx.enter_context(tc.tile_pool(name="o_sb_pool", bufs=3))
        small = ctx.enter_context(tc.tile_pool(name="small", bufs=4))
        fc_pool = ctx.enter_context(tc.tile_pool(name="fc_pool", bufs=2))

        # ---- constants / inputs (split DMAs, just-in-time order) ----
        wq_sb = consts.tile([128, 3, CIN_CH, C], bf16)
        wk_sb = consts.tile([128, 3, CIN_CH, C], bf16)
        wv_sb = consts.tile([128, 3, CIN_CH, C], bf16)
        wfc_sb = consts.tile([128, 3, CIN_CH, C], bf16)
        q_in = consts.tile([128, CIN_CH, QIN_W], bf16)
        k_in = consts.tile([128, CIN_CH, 1026], bf16)
        v_in = consts.tile([128, CIN_CH, 1026], bf16)

        def dma_w(sb, d):  # per-tap pieces so the first matmul starts early
            for t in range(3):
                nc.sync.dma_start(
                    out=sb[:, t], in_=d[t].rearrange("(ki p) co -> p ki co",
                                                     p=128))

        def dma_x(sb, d):  # per-cin-chunk pieces
            r = d.rearrange("(ki p) l -> ki p l", p=128)
            for ki in range(CIN_CH):
                nc.sync.dma_start(out=sb[:, ki], in_=r[ki])

        dma_w(wk_sb, wk_d)
        dma_x(k_in, k_in_d)
        dma_w(wv_sb, wv_d)
        dma_x(v_in, v_in_d)
        dma_w(wq_sb, wq_d)
        dma_x(q_in, q_in_d)
        dma_w(wfc_sb, wfc_d)

        mab_sb = consts.tile([128, 2], f32)
        nc.sync.dma_start(out=mab_sb, in_=bcast_rows(mab_d, 128))
        ident = consts.tile([128, 128], bf16)
        make_identity(nc, ident)

        qb_bc = kb_bc = vb_sb = fb_sb = None
        if use_qb:
            qb_bc = consts.tile([128, C], f32)
            nc.sync.dma_start(out=qb_bc, in_=bcast_rows(qb_d, 128))
        if use_kb:
            kb_bc = consts.tile([128, C], f32)
            nc.sync.dma_start(out=kb_bc, in_=bcast_rows(kb_d, 128))
        if use_vb:
            vb_sb = consts.tile([128, CIN_CH], f32)
            nc.sync.dma_start(out=vb_sb, in_=vb_d)
        if use_fb:
            fb_sb = consts.tile([128, CIN_CH], f32)
            nc.sync.dma_start(out=fb_sb, in_=fb_d)

        kT = consts.tile([128, KC, C], bf16)     # [l(16 chunks), c]
        kT_loc = consts.tile([128, 8, C], bf16)  # own half before exchange
        qT = consts.tile([128, 10, C], bf16)     # slots 0-7 own, 8/9 boundary
        # slot stride 65; padded past the last slot so mm2 can read a
        # 128-wide lhsT (FWL-eligible -> LDWEIGHTS hidden); the extra
        # columns only feed ignored PSUM rows 65..127
        v_slots = consts.tile([128, CIN_CH, 32 * VSLOT + 64], bf16)
        v_loc = consts.tile([128, CIN_CH, 16 * VSLOT], bf16)
        attn_out = consts.tile([128, CIN_CH, QW], bf16)
        nc.vector.memset(v_loc, 1.0)             # ones cols; data overwritten
        nc.vector.memset(v_slots[:, :, 32 * VSLOT:], 0.0)  # lhsT overread pad

        def conv_transposed(x_in, w_sb, bias_bc, out_sb, slot, col0):
            ps = conv_ps.tile([128, 512], f32, name="convps")
            n = 0
            for t in range(3):
                for ki in range(CIN_CH):
                    nc.tensor.matmul(
                        ps,
                        lhsT=x_in[:, ki, col0 + t: col0 + t + 128],
                        rhs=w_sb[:, t, ki, :],
                        start=(n == 0), stop=(n == 11))
                    n += 1
            dst = out_sb[:, slot, :]
            if bias_bc is not None:
                nc.vector.tensor_add(dst, ps, bias_bc)
            else:
                nc.vector.tensor_copy(dst, ps)

        # ---- k conv (transposed), own l-half only ----
        for s in range(8):
            conv_transposed(k_in, wk_sb, kb_bc if use_kb else None,
                            kT_loc, s, s * 128)
        nc.sync.dma_start(out=k_src, in_=kT_loc)
        nc.gpsimd.collective_compute(
            kind="AllGather", op=mybir.AluOpType.bypass,
            replica_groups=[[0, 1], [2, 3], [4, 5], [6, 7]],
            ins=[k_src], outs=[k_gath])
        nc.sync.dma_start(out=kT[:, 0:8, :], in_=k_gath[0])
        nc.sync.dma_start(out=kT[:, 8:16, :], in_=k_gath[1])

        # ---- v conv (normal) into slotted layout (own 2 j-groups) ----
        def v_tile(co, lt):
            ps = conv_ps.tile([128, 512], f32, name="convps")
            n = 0
            for t in range(3):
                for ki in range(CIN_CH):
                    nc.tensor.matmul(
                        ps,
                        lhsT=wv_sb[:, t, ki, co * 128:(co + 1) * 128],
                        rhs=v_in[:, ki, lt * 512 + t: lt * 512 + t + 512],
                        start=(n == 0), stop=(n == 11))
                    n += 1
            dst = v_loc[:, co, lt * 8 * VSLOT:(lt + 1) * 8 * VSLOT] \
                .rearrange("p (h e) -> p h e", e=VSLOT)[:, :, 0:D]
            src = ps.rearrange("p (h d) -> p h d", d=D)
            if use_vb:
                nc.vector.tensor_scalar_add(dst, src, vb_sb[:, co:co + 1])
            else:
                nc.vector.tensor_copy(dst, src)

        for co in range(COUT_CH):
            for lt in range(2):
                v_tile(co, lt)
        nc.sync.dma_start(out=v_src, in_=v_loc)
        nc.gpsimd.collective_compute(
            kind="AllGather", op=mybir.AluOpType.bypass,
            replica_groups=[[0, 1], [2, 3], [4, 5], [6, 7]],
            ins=[v_src], outs=[v_gath])
        nc.sync.dma_start(out=v_slots[:, :, 0:16 * VSLOT], in_=v_gath[0])
        nc.sync.dma_start(out=v_slots[:, :, 16 * VSLOT:32 * VSLOT],
                          in_=v_gath[1])

        # ---- q conv (transposed): slots 0-3 feed the jj=0 units; the
        # rest are emitted after the first unit so they fill PE gaps ----
        def q_slot(s, col0):
            conv_transposed(q_in, wq_sb, qb_bc if use_qb else None,
                            qT, s, col0)

        for s in range(4):
            q_slot(s, s * 128)

        # ---- attention ----
        def mm1_exp_round(h, rnd, qrow0, qslot, exp_t):
            p0 = (h % 2) * 64
            sc = scores_ps.tile([128, 2, 512], f32, name="sc")
            for jx in range(2):
                c2 = rnd * 2 + jx
                jp, cc = c2 // 4, c2 % 4
                nc.tensor.matmul(
                    sc[:, jx, :],
                    lhsT=kT[p0:p0 + 64, jp * 4 + h // 2,
                            cc * 128:(cc + 1) * 128],
                    rhs=qT[qrow0:qrow0 + 64, qslot, :],
                    start=True, stop=True)
            nc.scalar.activation(
                out=exp_t[:, rnd * 2:(rnd + 1) * 2, :], in_=sc,
                func=mybir.ActivationFunctionType.Exp, scale=1.0 / D)

        def finish_head(h, exp_t, out_cols, d0, d1, mask_idx=None):
            o = o_ps.tile([128, 512], f32, name="o")
            for c2 in range(KC):
                jp, cc = c2 // 4, c2 % 4
                base = (jp * 8 + h) * VSLOT
                nc.tensor.matmul(o, lhsT=v_slots[:, cc, base:base + 128],
                                 rhs=exp_t[:, c2, :],
                                 start=(c2 == 0), stop=(c2 == KC - 1))
            o_sb = o_sb_pool.tile([VSLOT, 512], bf16, name="o_sb")
            nc.vector.tensor_copy(o_sb, o[0:VSLOT, :])
            for cc in range(4):
                tp = tp_ps.tile([128, VSLOT], bf16, name="tp", tag="convps")
                nc.tensor.transpose(tp,
                                    o_sb[:, cc * 128:(cc + 1) * 128],
                                    ident[0:VSLOT, 0:VSLOT])
                rc = small.tile([128, 1], f32, name="rc")
                nc.vector.reciprocal(rc, tp[:, D:D + 1])
                dst = out_cols(cc)
                if mask_idx is None:
                    nc.vector.tensor_scalar_mul(dst, tp[:, d0:d1], rc)
                else:
                    nc.vector.tensor_scalar(
                        out=dst, in0=tp[:, d0:d1], scalar1=rc,
                        scalar2=mab_sb[:, mask_idx:mask_idx + 1],
                        op0=mybir.AluOpType.mult,
                        op1=mybir.AluOpType.mult)
            return o_sb

        def main_out_cols(h, jj):
            def out_cols(cc):
                lo = 1 + jj * 512 + h * D
                return attn_out[:, cc, lo:lo + D]
            return out_cols

        def pair_unit(m, jj):
            # Heads 2m / 2m+1 share kT slots; their mm1 lhsTs sit in
            # disjoint PE row groups (partitions 0-63 / 64-127) so the
            # interleaved matmuls run concurrently in the array.
            hA, hB = 2 * m, 2 * m + 1
            eA = exp_pool.tile([128, KC, 512], bf16, name="exp_t")
            eB = exp_pool.tile([128, KC, 512], bf16, name="exp_t")
            for rnd in range(8):
                mm1_exp_round(hA, rnd, 0, jj * 4 + m, eA)
                mm1_exp_round(hB, rnd, 64, jj * 4 + m, eB)
            oA = finish_head(hA, eA, main_out_cols(hA, jj), 0, D)
            finish_head(hB, eB, main_out_cols(hB, jj), 0, D)
            if dbg and m == 0 and jj == 0:
                nc.sync.dma_start(out=dbg_exp, in_=eA)
                nc.sync.dma_start(out=dbg_o, in_=oA)

        def variant(vi):
            var = VARIANTS[vi]

            def out_cols(cc, col=var["col"]):
                return attn_out[:, cc, col:col + 1]

            e = exp_pool.tile([128, KC, 512], bf16, name="exp_t")
            for rnd in range(8):
                mm1_exp_round(var["h"], rnd, var["qrow0"], var["qslot"], e)
            finish_head(var["h"], e, out_cols, var["d"], var["d"] + 1,
                        mask_idx=vi)

        def fc_tile(co, lt):
            ps = conv_ps.tile([128, 512], f32, name="convps")
            n = 0
            for t in range(3):
                for ki in range(CIN_CH):
                    nc.tensor.matmul(
                        ps,
                        lhsT=wfc_sb[:, t, ki, co * 128:(co + 1) * 128],
                        rhs=attn_out[:, ki, lt * 512 + t: lt * 512 + t + 512],
                        start=(n == 0), stop=(n == 11))
                    n += 1
            fc_sb = fc_pool.tile([128, 512], f32, name="fc_sb")
            if use_fb:
                nc.vector.tensor_scalar_add(fc_sb, ps, fb_sb[:, co:co + 1])
            else:
                nc.vector.tensor_copy(fc_sb, ps)
            nc.sync.dma_start(
                out=out_d[co * 128:(co + 1) * 128, lt * 512:(lt + 1) * 512],
                in_=fc_sb)

        # Remaining q-conv slots first: they fill PE time while the k/v
        # AllGather round-trips complete.
        for s in range(4, 8):
            q_slot(s, s * 128)
        q_slot(8, 1026)
        q_slot(9, 1156)
        pair_unit(0, 0)
        pair_unit(1, 0)
        pair_unit(2, 0)
        pair_unit(3, 0)
        variant(0)  # boundary col 0 (l'=1023)
        # jj=1 pairs interleaved with the first fc row (needs cols 0..513);
        # variant 1 early so its output (col 1025) is ready before fc lt=1
        for m in range(4):
            pair_unit(m, 1)
            if m == 0:
                variant(1)  # boundary col 1025 (l'=1024)
            fc_tile(m, 0)

        if dbg:
            for sb, dd in ((kT, dbg_kt), (qT, dbg_qt),
                           (v_slots[:, :, 0:32 * VSLOT], dbg_vs),
                           (attn_out, dbg_ao)):
                nc.sync.dma_start(out=dd, in_=sb)

        for co in range(COUT_CH):
            fc_tile(co, 1)

    nc.compile()
    return nc


def kernel(q, k, v, wq_w, wq_b, wk_w, wk_b, wv_w, wv_b, fc_w, fc_b):
    q = np.asarray(q, np.float32)
    k = np.asarray(k, np.float32)
    v = np.asarray(v, np.float32)
    wq_w = np.asarray(wq_w, np.float32)
    wk_w = np.asarray(wk_w, np.float32)
    wv_w = np.asarray(wv_w, np.float32)
    fc_w = np.asarray(fc_w, np.float32)
    wq_b = np.asarray(wq_b, np.float32)
    wk_b = np.asarray(wk_b, np.float32)
    wv_b = np.asarray(wv_b, np.float32)
    fc_b = np.asarray(fc_b, np.float32)

    flags = (bool(wq_b.any()), bool(wk_b.any()),
             bool(wv_b.any()), bool(fc_b.any()))
    if flags not in _CACHE:
        _CACHE[flags] = _build(flags)
    nc = _CACHE[flags]
    use_qb, use_kb, use_vb, use_fb = flags

    def prep_w(w):  # [Cout, Cin, 3] -> [3, Cin, Cout]
        return np.ascontiguousarray(w.transpose(2, 1, 0)).astype(BF16)

    wq_t, wk_t, wv_t, wfc_t = map(prep_w, (wq_w, wk_w, wv_w, fc_w))

    in_maps = []
    for core in range(NCORES):
        b, half = core // 2, core % 2
        qlo = half * HALF
        qpad = np.zeros((C, L + 2), np.float32)
        qpad[:, 1:L + 1] = q[b]
        # seg1: own window l in [qlo-1, qlo+1025); seg2a: l 895..1025
        # (slot 8, outputs l 896..1024); seg2b: l 1023..1153 (slot 9)
        q_in = np.concatenate(
            [qpad[:, qlo:qlo + 1026], qpad[:, 896:1026],
             qpad[:, 1024:1154]], axis=1)
        kpad = np.zeros((C, L + 2), np.float32)
        kpad[:, 1:L + 1] = k[b]
        vpad = np.zeros((C, L + 2), np.float32)
        vpad[:, 1:L + 1] = v[b]
        m = {
            "q_in": q_in.astype(BF16),
            "k_in": kpad[:, qlo:qlo + 1026].astype(BF16),
            "v_in": vpad[:, qlo:qlo + 1026].astype(BF16),
            "wq": wq_t, "wk": wk_t, "wv": wv_t, "wfc": wfc_t,
            # mab[0] gates buffer col 0 (l'=1023, valid for half=1);
            # mab[1] gates col 1025 (l'=1024, valid for half=0)
            "mab": np.array([[float(half == 1), float(half == 0)]],
                            np.float32),
        }
        if use_qb:
            m["qb"] = wq_b.reshape(1, C)
        if use_kb:
            m["kb"] = wk_b.reshape(1, C)
        if use_vb:
            m["vb"] = np.ascontiguousarray(wv_b.reshape(CIN_CH, 128).T)
        if use_fb:
            m["fb"] = np.ascontiguousarray(fc_b.reshape(CIN_CH, 128).T)
        in_maps.append(m)

    global _LAST_IN_MAPS, _LAST_RESULTS
    _LAST_IN_MAPS = in_maps
    from concourse.bass_utils import run_bass_kernel_spmd
    res = run_bass_kernel_spmd(nc, in_maps, list(range(NCORES))).results
    _LAST_RESULTS = res

    out = np.empty((B, C, L), np.float32)
    for core in range(NCORES):
        b, half = core // 2, core % 2
        out[b][:, half * HALF:(half + 1) * HALF] = res[core]["out"]
    return out
